# revision 20
# baseline (speedup 1.0000x reference)
"""Trainium2 Bass kernel for GNO message passing — v4 (ragged tail).

Same architecture as v3 (fp16 matmuls, [120, 3*w] ACT windows, software-
pipelined ACT/PE streams), with:
  - Ragged last supertile (144 cols vs 512) so per-core padding drops
    from 11,264 edges to 224: saves ~1.9us of ACT busy and shortens the
    pipeline drain.
  - Per-supertile input fetches, first fetch issued before the weight
    DMAs so L1(0)/A1(0) start earlier.
"""

import numpy as np

import concourse.bass as bass
import concourse.mybir as mybir
from concourse.bacc import Bacc
from concourse.tile import TileContext
from concourse.bass_utils import run_bass_kernel_spmd

# Problem sizes (hardcoded per contract)
N_S = 131072
N_D = 131072
E = 8388608
DIM = 3
H = 12

N_CORES = 8
S = 10            # streams (12-row hidden blocks) per column
B = 3             # 32-row quadrants per supertile
T = 512           # max columns per quadrant per supertile (1 PSUM bank fp32)
# narrow head supertiles shorten the serialized pipeline-fill chain;
# ragged tail trims edge padding. sum(WIDTHS)*30 >= E/8.
WIDTHS = [448, 464] + [T] * 66 + [256]
NST = len(WIDTHS)
OFF = np.concatenate([[0], np.cumsum(WIDTHS)]).astype(int)
W = int(OFF[-1])            # relin/kout columns per core = 34960
R = 32 * B                  # device tensor rows = 96
E_PC = W * B * S            # edges per core (padded) = 1,048,800
E_PAD = N_CORES * E_PC

_BASS_CACHE = {}


def _build_bass():
    if "nc" in _BASS_CACHE:
        return _BASS_CACHE["nc"]
    fp32 = mybir.dt.float32
    fp16 = mybir.dt.float16
    GELU = mybir.ActivationFunctionType.Gelu
    nc = Bacc()

    relin = nc.dram_tensor("relin", [R, W], fp16, kind="ExternalInput")
    # wtabw cols (fp16): [0:120]=w1s (3 copies, rows 32m:32m+30),
    # [120:240]=w2, [240:272]=w3e (32 cols, last 2 zero)
    wtabw = nc.dram_tensor("wtabw", [128, 272], fp16, kind="ExternalInput")
    # wtabb cols (fp32): 0=b1, 1=b2 (rows 0:120), 2=b3 (rows 0:96)
    wtabb = nc.dram_tensor("wtabb", [128, 3], fp32, kind="ExternalInput")
    kout = nc.dram_tensor("kout", [R, W], fp16, kind="ExternalOutput")

    with TileContext(nc) as tc:
        with (
            tc.tile_pool(name="wpool", bufs=1) as wpool,
            tc.tile_pool(name="inpool", bufs=3) as inpool,
            tc.tile_pool(name="g1pool", bufs=2) as g1pool,
            tc.tile_pool(name="g2pool", bufs=2) as g2pool,
            tc.tile_pool(name="kspool", bufs=2) as kspool,
            tc.tile_pool(name="ph1", bufs=1, space="PSUM") as ph1,
            tc.tile_pool(name="ph2", bufs=1, space="PSUM") as ph2,
            tc.tile_pool(name="pk", bufs=2, space="PSUM") as pk,
        ):
            # Dummy 1x1 GELU: forces the ACT-table load (~1.3us) to run at
            # t=0, overlapped with the prologue DMAs, instead of right
            # before the first real GELU.
            warm_in = wpool.tile([1, 1], fp32, tag="warm_in")
            warm_out = wpool.tile([1, 1], fp32, tag="warm_out")
            nc.gpsimd.memset(warm_in[:], 0.0)
            nc.scalar.activation(warm_out[:], warm_in[:], GELU)

            def fetch(t, eng=None):
                w = WIDTHS[t]
                xch = inpool.tile([R, T], fp16, tag="xin")
                (eng or nc.sync).dma_start(
                    xch[:, 0:w], relin[:, OFF[t]:OFF[t] + w])
                return xch

            xch0 = fetch(0, eng=nc.gpsimd)

            wtab_sb = wpool.tile([128, 272], fp16, tag="wtabw")
            nc.sync.dma_start(wtab_sb[:], wtabw[:, :])
            wb_sb = wpool.tile([128, 3], fp32, tag="wtabb")
            nc.sync.dma_start(wb_sb[:], wtabb[:, :])
            w1s = [wtab_sb[32 * m:32 * m + 30, 0:120] for m in range(B)]
            w2 = wtab_sb[0:120, 120:240]
            w3e = wtab_sb[0:120, 240:272]
            b1 = wb_sb[0:120, 0:1]
            b2 = wb_sb[0:120, 1:2]
            b3 = wb_sb[0:R, 2:3]

            def l1(t, xch):
                w = WIDTHS[t]
                h1 = ph1.tile([120, B * T], fp32, tag="h1")
                for m in range(B):
                    # quadrants at fixed stride T so each PSUM write stays
                    # inside one bank even for ragged widths
                    nc.tensor.matmul(
                        h1[:, T * m:T * m + w], w1s[m],
                        xch[32 * m:32 * m + 30, 0:w],
                        start=True, stop=True, tile_position=(32 * m, 0))
                return h1

            # state carried across pipeline stages
            h1_t = h2_t = h2g_t = None
            h1_t = l1(0, xch0)

            for t in range(NST + 1):
                # ACT stream: A1(t), A2(t-1)
                if t < NST:
                    w = WIDTHS[t]
                    h1g = g1pool.tile([120, B * T], fp16, tag="h1g")
                    nc.scalar.activation(h1g[:, 0:2 * T + w],
                                         h1_t[:, 0:2 * T + w], GELU, bias=b1)
                if t >= 1:
                    wp = WIDTHS[t - 1]
                    h2g_t = g2pool.tile([120, B * T], fp16, tag="h2g")
                    nc.scalar.activation(h2g_t[:, 0:2 * T + wp],
                                         h2_t[:, 0:2 * T + wp], GELU, bias=b2)

                # PE stream: L1(t+1), L2(t), L3(t-1)
                if t + 1 < NST:
                    h1_t = l1(t + 1, fetch(
                        t + 1, eng=nc.gpsimd if t + 1 <= 3 else None))
                if t < NST:
                    h2 = ph2.tile([120, B * T], fp32, tag="h2")
                    for m in range(B):
                        nc.tensor.matmul(
                            h2[:, T * m:T * m + w], w2,
                            h1g[:, T * m:T * m + w],
                            start=True, stop=True)
                if t >= 1:
                    kt = pk.tile([R, T], fp32, tag="kt")
                    for m in range(B):
                        nc.tensor.matmul(
                            kt[32 * m:32 * m + 32, 0:wp], w3e,
                            h2g_t[:, T * m:T * m + wp],
                            start=True, stop=True, tile_position=(0, 32 * m))
                    ksb = kspool.tile([R, T], fp16, tag="ksb")
                    nc.vector.tensor_scalar_add(ksb[:, 0:wp], kt[:, 0:wp], b3)
                    out_eng = nc.sync if t >= NST - 3 else nc.gpsimd
                    out_eng.dma_start(
                        kout[:, OFF[t - 1]:OFF[t]], ksb[:, 0:wp])

                if t < NST:
                    h2_t = h2

    nc.finalize()
    _BASS_CACHE["nc"] = nc
    return nc


def _erf(x):
    # Abramowitz & Stegun 7.1.26 (|err| <= 1.5e-7)
    a1, a2, a3, a4, a5 = (0.254829592, -0.284496736, 1.421413741,
                          -1.453152027, 1.061405429)
    p = 0.3275911
    s = np.sign(x)
    ax = np.abs(x)
    t = 1.0 / (1.0 + p * ax)
    y = 1.0 - (((((a5 * t + a4) * t) + a3) * t + a2) * t + a1) * t * np.exp(-ax * ax)
    return s * y

try:
    from scipy.special import erf as _erf  # noqa: F811
except Exception:
    pass


def _gelu_np(x):
    return 0.5 * x * (1.0 + _erf(x / np.sqrt(2.0)))


# per-supertile edge offsets (edges ordered (t, m, col, s))
E_OFF = [int(OFF[t]) * B * S for t in range(NST + 1)]


def _pack_inputs(x_sparse, f_sparse, x_dense, W1, b1, W2, b2, W3, b3,
                 edge_src, edge_dst):
    src = np.asarray(edge_src).astype(np.int64)
    dst = np.asarray(edge_dst).astype(np.int64)
    x_sparse = np.asarray(x_sparse, dtype=np.float32)
    x_dense = np.asarray(x_dense, dtype=np.float32)

    RELP = np.zeros((E_PAD, DIM), np.float32)
    RELP[:E] = x_sparse[src]
    RELP[:E] -= x_dense[dst]

    W1 = np.asarray(W1, np.float32)
    W2 = np.asarray(W2, np.float32)
    W3 = np.asarray(W3, np.float32)
    wtabw = np.zeros((128, 272), np.float32)
    for m in range(B):
        for s in range(S):
            wtabw[32 * m + 3 * s:32 * m + 3 * s + 3, 12 * s:12 * s + 12] = W1
    for s in range(S):
        wtabw[12 * s:12 * s + 12, 120 + 12 * s:120 + 12 * s + 12] = W2
        wtabw[12 * s:12 * s + 12, 240 + 3 * s:240 + 3 * s + 3] = W3
    wtabb = np.zeros((128, 3), np.float32)
    wtabb[0:120, 0] = np.tile(np.asarray(b1, np.float32), S)
    wtabb[0:120, 1] = np.tile(np.asarray(b2, np.float32), S)
    b3q = np.zeros(32, np.float32)
    b3q[:30] = np.tile(np.asarray(b3, np.float32), S)
    wtabb[0:R, 2] = np.tile(b3q, B)

    in_maps = []
    for c in range(N_CORES):
        r = RELP[c * E_PC:(c + 1) * E_PC]
        rq = np.zeros((B, 32, W), np.float32)
        # per supertile: (m, col, s, dim) -> row 32m+3s+j, col OFF[t]+col
        for t in range(NST):
            wt = WIDTHS[t]
            seg = r[E_OFF[t]:E_OFF[t + 1]].reshape(B, wt, S, DIM)
            rq[:, :30, OFF[t]:OFF[t + 1]] = (
                seg.transpose(0, 2, 3, 1).reshape(B, 30, wt))
        in_maps.append({
            "relin": rq.reshape(R, W).astype(np.float16),
            "wtabw": wtabw.astype(np.float16),
            "wtabb": wtabb,
        })
    return in_maps, src, dst


def _host_tail(outs, f_sparse, src, dst, P1w, P1b, P2w, P2b, P3w, P3b):
    f_sparse = np.asarray(f_sparse, dtype=np.float32)
    msg = np.empty((E_PAD, DIM), np.float32)
    for c in range(N_CORES):
        ko = np.asarray(outs[c]["kout"]).astype(np.float32)
        ko = ko.reshape(B, 32, W)[:, :30]              # [B, 30, W]
        dstc = msg[c * E_PC:(c + 1) * E_PC]
        for t in range(NST):
            wt = WIDTHS[t]
            seg = ko[:, :, OFF[t]:OFF[t + 1]].reshape(B, S, DIM, wt)
            dstc[E_OFF[t]:E_OFF[t + 1]] = (
                seg.transpose(0, 3, 1, 2).reshape(wt * B * S, DIM))
    msg = msg[:E] * f_sparse[src]

    cnt = np.bincount(dst, minlength=N_D).astype(np.float32)
    starts = (np.cumsum(cnt) - cnt).astype(np.int64)
    nz = cnt > 0
    sums = np.zeros((N_D, DIM), np.float32)
    if nz.any():
        sums[nz] = np.add.reduceat(msg, starts[nz], axis=0)
    out_feat = sums / np.maximum(cnt, 1.0)[:, None]

    h = _gelu_np(out_feat.astype(np.float64) @ np.asarray(P1w, np.float64)
                 + np.asarray(P1b, np.float64))
    h = _gelu_np(h @ np.asarray(P2w, np.float64) + np.asarray(P2b, np.float64))
    out = h @ np.asarray(P3w, np.float64) + np.asarray(P3b, np.float64)
    return out.astype(np.float32)


def kernel(x_sparse, f_sparse, x_dense, W1, b1, W2, b2, W3, b3,
           P1w, P1b, P2w, P2b, P3w, P3b, edge_src, edge_dst):
    in_maps, src, dst = _pack_inputs(x_sparse, f_sparse, x_dense, W1, b1,
                                     W2, b2, W3, b3, edge_src, edge_dst)
    nc = _build_bass()
    res = None
    for attempt in range(3):
        try:
            res = run_bass_kernel_spmd(nc, in_maps, list(range(N_CORES)))
            break
        except Exception:
            # transient NRT_EXEC_UNIT_UNRECOVERABLE wedges recover on retry
            if attempt == 2:
                raise
            import time as _time
            _time.sleep(2.0 * (attempt + 1))
    return _host_tail(res.results, f_sparse, src, dst,
                      P1w, P1b, P2w, P2b, P3w, P3b)


def run_profiled(inputs, tmpdir=None):
    """Run once with tracing enabled; returns BassKernelResults."""
    kw = {k: v for k, v in inputs.items()
          if k in ("x_sparse", "f_sparse", "x_dense", "W1", "b1", "W2", "b2",
                   "W3", "b3", "edge_src", "edge_dst")}
    in_maps, _, _ = _pack_inputs(**kw)
    nc = _build_bass()
    return run_bass_kernel_spmd(nc, in_maps, list(range(N_CORES)),
                                trace=True, tmpdir=tmpdir)
